# revision 4
# baseline (speedup 1.0000x reference)
"""Trainium2 Bass kernel for PointSegBatchlossHead (3-NN interpolation + MLP heads).

Strategy: data-parallel over frames/points across 8 NeuronCores.
Cores 0-3 handle frame 0, cores 4-7 frame 1; each core takes 5000 of the
frame's 20000 query points (full 10000-reference KNN per frame is local),
plus 2500 rows of the conv-cls head. MLP weights are replicated.

Per query tile of 128 points the core computes s = q.r - |r|^2/2 for all
10000 refs with fp32 PE matmuls (K=4: [qx,qy,qz,1] x [rx,ry,rz,-|r|^2/2]),
takes top-8 via the DVE max/max_index select ops (argmax s == argmin d2),
gathers the 3 nearest feature rows with a GPSIMD DMA gather, and applies
inverse-distance weights (d2 = |q|^2 - 2s) followed by the align/out MLP
head evaluated in channels-on-partitions layout (PE transposes on entry
and exit).
"""

import sys

sys.path.insert(0, "/opt/trn_rl_repo")

import numpy as np

import bass_rust
import concourse.bass as bass
import concourse.tile as tile
from concourse import bacc, library_config, mybir
from concourse.vector_clock import ScopedClock

f32 = mybir.dt.float32
u16 = mybir.dt.uint16
i16 = mybir.dt.int16

B, NC, NP = 2, 10000, 20000
CIN, HID, NCLS, K = 64, 64, 20, 3
NCORES = 8
CORES_PER_FRAME = 4
QSH = NP // CORES_PER_FRAME          # 5000 queries per core
QPAD = 5120                          # 40 tiles of 128
NQT = QPAD // 128                    # 40
CSH = (B * NC) // NCORES             # 2500 conv rows per core
CPAD = 2560
NCT = CPAD // 128                    # 20
RTILES = 79                          # ceil(10000/128) ref row tiles
RPAD = RTILES * 128                  # 10112
PSG = 1536                           # psum group width (3 banks)
MMCH = 512                           # matmul moving free dim

EPS_BN_CLS = 1e-5
EPS_BN_ALIGN = 1e-6
EPS_NN = 1e-8


def _split_sync_waits(nc, max_waits=1):
    """This walrus build accepts only one sync wait per instruction; hoist
    extras onto preceding NoOps on the same engine."""
    for bb in nc.main_func.blocks:
        new_list = []
        for inst in bb.instructions:
            si = inst.sync_info
            if si is not None and len(si.on_wait) > max_waits:
                waits = list(si.on_wait)
                extra, keep = waits[:-max_waits], waits[-max_waits:]
                for j in range(0, len(extra), max_waits):
                    nop = mybir.InstNoOp(name=f"{inst.name}-sw{j}", ins=[], outs=[])
                    nop.engine = inst.engine
                    nop.sync_info = bass_rust.SyncInfo(
                        on_wait=extra[j:j + max_waits], on_update=[])
                    try:
                        nc.register_instruction(nop, overwrite=True)
                    except Exception:
                        pass
                    new_list.append(nop)
                inst.sync_info = bass_rust.SyncInfo(
                    on_wait=keep, on_update=list(si.on_update))
            new_list.append(inst)
        bb.instructions = new_list


def _patch_drain():
    """Split the Tile end-of-kernel drain's aggregated sem waits (same
    1-wait-per-instruction walrus limitation)."""
    def patched(self, tick_clock, wait_clock):
        nc = self.nc
        drain_inst = nc.sync.drain()
        wait_clock.add_sem_waits(
            drain_inst.ins, ScopedClock({None: tick_clock.global_clock}))
        si = drain_inst.ins.sync_info
        waits = list(si.on_wait)
        if len(waits) > 1:
            drain_inst.ins.sync_info = bass_rust.SyncInfo(
                on_wait=waits[:1], on_update=list(si.on_update))
            for i in range(1, len(waits)):
                extra = nc.sync.drain()
                extra.ins.sync_info = bass_rust.SyncInfo(
                    on_wait=waits[i:i + 1], on_update=[])
        nc.all_engine_barrier()
        assert self.sems is not None
        popped = nc._tile_sem_poison_stack.pop()
        assert popped is self._sem_poison
        nc.clear_and_free_semaphores(list(self.sems.allocated().values()))
        nc.all_engine_barrier()
    tile.TileContext._drain_and_barrier = patched


def build_nc():
    _patch_drain()
    nc = bacc.Bacc(None)

    qT = nc.dram_tensor("qT", [4, QPAD], f32, kind="ExternalInput")
    q_rows = nc.dram_tensor("q_rows", [QPAD, 4], f32, kind="ExternalInput")
    rT = nc.dram_tensor("rT", [3, NC], f32, kind="ExternalInput")
    r_rows = nc.dram_tensor("r_rows", [RPAD, 4], f32, kind="ExternalInput")
    F = nc.dram_tensor("F", [NC, CIN], f32, kind="ExternalInput")
    Fc = nc.dram_tensor("Fc", [CPAD, CIN], f32, kind="ExternalInput")
    ident = nc.dram_tensor("ident", [128, 128], f32, kind="ExternalInput")
    cls_w1 = nc.dram_tensor("cls_w1", [CIN, HID], f32, kind="ExternalInput")
    cls_w2 = nc.dram_tensor("cls_w2", [HID, NCLS], f32, kind="ExternalInput")
    al_w = nc.dram_tensor("al_w", [CIN, HID], f32, kind="ExternalInput")
    out_w1 = nc.dram_tensor("out_w1", [HID, HID], f32, kind="ExternalInput")
    out_w2 = nc.dram_tensor("out_w2", [HID, NCLS], f32, kind="ExternalInput")
    # packed per-channel vectors: col 0 cls_scale, 1 cls_bias, 2 cls_b2,
    # 3 al_scale, 4 al_bias_folded, 5 out_scale, 6 out_bias, 7 out_b2
    wv = nc.dram_tensor("wv", [64, 8], f32, kind="ExternalInput")

    out_logits = nc.dram_tensor("out_logits", [QPAD, NCLS], f32, kind="ExternalOutput")
    conv_logits = nc.dram_tensor("conv_logits", [CPAD, NCLS], f32, kind="ExternalOutput")

    with tile.TileContext(nc) as tc:
        with (
            tc.tile_pool(name="sbig", bufs=2) as sbig,
            tc.tile_pool(name="refs", bufs=1) as refs,
            tc.tile_pool(name="qts", bufs=1) as qts,
            tc.tile_pool(name="wts", bufs=1) as wts,
            tc.tile_pool(name="work", bufs=3) as work,
            tc.tile_pool(name="ps_s", bufs=2, space="PSUM") as ps_s,
            tc.tile_pool(name="ps_h", bufs=2, space="PSUM") as ps_h,
            tc.tile_pool(name="dram", bufs=3, space="DRAM") as dram,
        ):
            nc.gpsimd.load_library(library_config.mlp)

            # ---------------- prep: weights ----------------
            w_cls1 = wts.tile([CIN, HID], f32, tag="w1")
            nc.sync.dma_start(w_cls1[:, :], cls_w1[:, :])
            w_cls2 = wts.tile([HID, NCLS], f32, tag="w2")
            nc.sync.dma_start(w_cls2[:, :], cls_w2[:, :])
            w_al = wts.tile([CIN, HID], f32, tag="w3")
            nc.sync.dma_start(w_al[:, :], al_w[:, :])
            w_o1 = wts.tile([HID, HID], f32, tag="w4")
            nc.sync.dma_start(w_o1[:, :], out_w1[:, :])
            w_o2 = wts.tile([HID, NCLS], f32, tag="w5")
            nc.sync.dma_start(w_o2[:, :], out_w2[:, :])
            wvs = wts.tile([64, 8], f32, tag="wv")
            nc.sync.dma_start(wvs[:, :], wv[:, :])
            idn = wts.tile([128, 128], f32, tag="idn")
            nc.sync.dma_start(idn[:, :], ident[:, :])

            # ---------------- prep: refs rT4 = [rx,ry,rz,-|r|^2/2] ----------------
            rT4 = refs.tile([4, NC], f32, tag="rT4")
            nc.sync.dma_start(rT4[0:3, :], rT[:, :])
            wide = refs.tile([128, RTILES, 4], f32, tag="wide")
            nc.sync.dma_start(
                wide[:, :, :], r_rows[:, :].rearrange("(t p) c -> p t c", p=128))
            sqw = refs.tile([128, RTILES, 4], f32, tag="sqw")
            nc.vector.tensor_tensor(
                out=sqw[:, :, :], in0=wide[:, :, :], in1=wide[:, :, :],
                op=mybir.AluOpType.mult)
            nsum = refs.tile([128, RTILES], f32, tag="nsum")
            nc.vector.tensor_tensor(
                out=nsum[:, :], in0=sqw[:, :, 1], in1=sqw[:, :, 2],
                op=mybir.AluOpType.add)
            nc.vector.tensor_tensor(
                out=nsum[:, :], in0=nsum[:, :], in1=sqw[:, :, 3],
                op=mybir.AluOpType.add)
            nc.vector.tensor_scalar_mul(nsum[:, :], nsum[:, :], -0.5)
            norms_d = dram.tile([RPAD], f32, tag="norms")
            nc.sync.dma_start(norms_d[:].rearrange("(t p) -> p t", p=128), nsum[:, :])
            nc.sync.dma_start(rT4[3:4, 0:NC], norms_d[0:NC][None, :])

            # ---------------- prep: queries ----------------
            qTs = qts.tile([4, QPAD], f32, tag="qT")
            nc.sync.dma_start(qTs[:, :], qT[:, :])

            # ---------------- conv cls head (emitted interleaved) ----------------
            def conv_tile(t):
                r0 = t * 128
                fc = work.tile([128, CIN], f32, tag="fc")
                nc.sync.dma_start(fc[:, :], Fc[r0:r0 + 128, :])
                ps_t = ps_h.tile([CIN, 128], f32, tag="ph")
                nc.tensor.transpose(ps_t[:, :], fc[:, :], idn[:, :])
                fcT = work.tile([CIN, 128], f32, tag="fcT")
                nc.scalar.copy(fcT[:, :], ps_t[:, :])

                ps_a = ps_h.tile([HID, 128], f32, tag="ph")
                nc.tensor.matmul(ps_a[:, :], w_cls1[:, :], fcT[:, :],
                                 start=True, stop=True)
                hT = work.tile([HID, 128], f32, tag="chT")
                nc.scalar.activation(
                    hT[:, :], ps_a[:, :], mybir.ActivationFunctionType.Relu,
                    bias=wvs[:, 1:2], scale=wvs[:, 0:1])

                ps_o = ps_h.tile([NCLS, 128], f32, tag="ph")
                nc.tensor.matmul(ps_o[:, :], w_cls2[:, :], hT[:, :],
                                 start=True, stop=True)
                oT = work.tile([NCLS, 128], f32, tag="coT")
                nc.scalar.activation(
                    oT[:, :], ps_o[:, :], mybir.ActivationFunctionType.Identity,
                    bias=wvs[0:NCLS, 2:3], scale=1.0)

                ps_f = ps_h.tile([128, NCLS], f32, tag="ph")
                nc.tensor.transpose(ps_f[:, :], oT[:, :], idn[0:NCLS, 0:NCLS])
                orow = work.tile([128, NCLS], f32, tag="corow")
                nc.scalar.copy(orow[:, :], ps_f[:, :])
                nc.sync.dma_start(conv_logits[r0:r0 + 128, :], orow[:, :])

            # ---------------- main loop over query tiles ----------------
            for t in range(NQT):
                if t % 2 == 0 and t // 2 < NCT:
                    conv_tile(t // 2)
                q0 = t * 128
                # s matrix for 128 queries x all refs
                s_tile = sbig.tile([128, NC], f32, tag="s")
                col = 0
                while col < NC:
                    gw = min(PSG, NC - col)
                    ps = ps_s.tile([128, PSG], f32, tag="ps")
                    sub = 0
                    while sub < gw:
                        n = min(MMCH, gw - sub)
                        nc.tensor.matmul(
                            ps[:, sub:sub + n],
                            qTs[:, q0:q0 + 128],
                            rT4[:, col + sub:col + sub + n],
                            start=True, stop=True)
                        sub += n
                    nc.scalar.copy(s_tile[:, col:col + gw], ps[:, 0:gw])
                    col += gw

                # |q|^2 per query
                qr = work.tile([128, 4], f32, tag="qr")
                nc.sync.dma_start(qr[:, :], q_rows[q0:q0 + 128, :])
                qsq = work.tile([128, 3], f32, tag="qsq")
                q2 = work.tile([128, 1], f32, tag="q2")
                nc.scalar.activation(
                    qsq[:, :], qr[:, 1:4], mybir.ActivationFunctionType.Square,
                    bias=0.0, scale=1.0, accum_out=q2[:, :])

                # top-8 (by s) + indices
                v8 = work.tile([128, 8], f32, tag="v8")
                nc.vector.max(v8[:, :], s_tile[:, :])
                idx8 = work.tile([128, 8], u16, tag="idx8")
                nc.vector.max_index(idx8[:, :], v8[:, :], s_tile[:, :])

                # weights: d = max(q2 - 2 s, 0) + eps ; w = 1/d ; w /= sum(w)
                d3 = work.tile([128, K], f32, tag="d3")
                nc.scalar.activation(
                    d3[:, :], v8[:, 0:K], mybir.ActivationFunctionType.Identity,
                    bias=q2[:, :], scale=-2.0)
                nc.vector.tensor_scalar(
                    out=d3[:, :], in0=d3[:, :], scalar1=0.0, scalar2=EPS_NN,
                    op0=mybir.AluOpType.max, op1=mybir.AluOpType.add)
                w3u = work.tile([128, K], f32, tag="w3u")
                wsum = work.tile([128, 1], f32, tag="wsum")
                nc.vector.reciprocal(w3u[:, :], d3[:, :])
                nc.scalar.activation(
                    w3u[:, :], w3u[:, :], mybir.ActivationFunctionType.Identity,
                    bias=0.0, scale=1.0, accum_out=wsum[:, :])
                rws = work.tile([128, 1], f32, tag="rws")
                nc.vector.reciprocal(rws[:, :], wsum[:, :])
                w3 = work.tile([128, K], f32, tag="w3")
                nc.scalar.activation(
                    w3[:, :], w3u[:, :], mybir.ActivationFunctionType.Copy,
                    bias=0.0, scale=rws[:, 0:1])

                # gather the 3 nearest feature rows per query
                idxd = dram.tile([128 * K], u16, tag="idxd")
                nc.sync.dma_start(
                    idxd[:].rearrange("(p k) -> p k", p=128), idx8[:, 0:K])
                idx_sb = work.tile([128, 8 * K], i16, tag="idxsb")
                isrc = idxd[:].bitcast(i16).rearrange("(c p k) -> p k c", p=16, k=K)
                for g in range(8):
                    dstg = idx_sb[16 * g:16 * g + 16, :].rearrange(
                        "p (k c) -> p k c", k=K)
                    nc.sync.dma_start(dstg[:, :, :], isrc[:, :, :])
                gat = work.tile([128, K, CIN], f32, tag="gat")
                nc.gpsimd.dma_gather(
                    out_ap=gat[:, :, :], in_ap=F[:, :], idxs_ap=idx_sb[:, :],
                    num_idxs=128 * K, num_idxs_reg=128 * K, elem_size=CIN)

                # interp = sum_k w_k * F_k
                m0 = work.tile([128, CIN], f32, tag="m0")
                nc.scalar.activation(
                    m0[:, :], gat[:, 0, :], mybir.ActivationFunctionType.Copy,
                    bias=0.0, scale=w3[:, 0:1])
                m1 = work.tile([128, CIN], f32, tag="m1")
                nc.scalar.activation(
                    m1[:, :], gat[:, 1, :], mybir.ActivationFunctionType.Copy,
                    bias=0.0, scale=w3[:, 1:2])
                m2 = work.tile([128, CIN], f32, tag="m2")
                nc.scalar.activation(
                    m2[:, :], gat[:, 2, :], mybir.ActivationFunctionType.Copy,
                    bias=0.0, scale=w3[:, 2:3])
                interp = work.tile([128, CIN], f32, tag="interp")
                nc.gpsimd.tensor_tensor(
                    out=interp[:, :], in0=m0[:, :], in1=m1[:, :],
                    op=mybir.AluOpType.add)
                nc.gpsimd.tensor_tensor(
                    out=interp[:, :], in0=interp[:, :], in1=m2[:, :],
                    op=mybir.AluOpType.add)

                # heads in channels-on-partitions layout
                ps_t = ps_h.tile([CIN, 128], f32, tag="ph")
                nc.tensor.transpose(ps_t[:, :], interp[:, :], idn[:, :])
                itT = work.tile([CIN, 128], f32, tag="itT")
                nc.scalar.copy(itT[:, :], ps_t[:, :])

                ps_a = ps_h.tile([HID, 128], f32, tag="ph")
                nc.tensor.matmul(ps_a[:, :], w_al[:, :], itT[:, :],
                                 start=True, stop=True)
                fT = work.tile([HID, 128], f32, tag="fT")
                nc.scalar.activation(
                    fT[:, :], ps_a[:, :], mybir.ActivationFunctionType.Relu,
                    bias=wvs[:, 4:5], scale=wvs[:, 3:4])

                ps_b = ps_h.tile([HID, 128], f32, tag="ph")
                nc.tensor.matmul(ps_b[:, :], w_o1[:, :], fT[:, :],
                                 start=True, stop=True)
                hT = work.tile([HID, 128], f32, tag="hT")
                nc.scalar.activation(
                    hT[:, :], ps_b[:, :], mybir.ActivationFunctionType.Relu,
                    bias=wvs[:, 6:7], scale=wvs[:, 5:6])

                ps_o = ps_h.tile([NCLS, 128], f32, tag="ph")
                nc.tensor.matmul(ps_o[:, :], w_o2[:, :], hT[:, :],
                                 start=True, stop=True)
                oT = work.tile([NCLS, 128], f32, tag="oT")
                nc.scalar.activation(
                    oT[:, :], ps_o[:, :], mybir.ActivationFunctionType.Identity,
                    bias=wvs[0:NCLS, 7:8], scale=1.0)

                ps_f = ps_h.tile([128, NCLS], f32, tag="ph")
                nc.tensor.transpose(ps_f[:, :], oT[:, :], idn[0:NCLS, 0:NCLS])
                orow = work.tile([128, NCLS], f32, tag="orow")
                nc.scalar.copy(orow[:, :], ps_f[:, :])
                nc.sync.dma_start(out_logits[q0:q0 + 128, :], orow[:, :])

    nc.compile()
    _split_sync_waits(nc)
    bass.Bass.finalize(nc)
    return nc


_NC = None


def _get_nc():
    global _NC
    if _NC is None:
        _NC = build_nc()
    return _NC


def _prep_in_maps(conv_point_features, conv_point_coords, points):
    conv_point_features = np.ascontiguousarray(conv_point_features, np.float32)
    conv_point_coords = np.ascontiguousarray(conv_point_coords, np.float32)
    points = np.ascontiguousarray(points, np.float32)

    ident = np.eye(128, dtype=np.float32)

    frames = []
    for f in range(B):
        rr = conv_point_coords[f * NC:(f + 1) * NC]       # [NC, 4]
        r_rows = np.zeros((RPAD, 4), np.float32)
        r_rows[:NC] = rr
        rT = np.ascontiguousarray(rr[:, 1:4].T)           # [3, NC]
        F = conv_point_features[f * NC:(f + 1) * NC]      # [NC, 64]
        frames.append((r_rows, rT, F))

    in_maps = []
    for c in range(NCORES):
        f = c // CORES_PER_FRAME
        qoff = (c % CORES_PER_FRAME) * QSH
        qr = points[f * NP + qoff:f * NP + qoff + QSH]    # [QSH, 4]
        q_rows = np.zeros((QPAD, 4), np.float32)
        q_rows[:QSH] = qr
        qT = np.zeros((4, QPAD), np.float32)
        qT[0:3, :QSH] = qr[:, 1:4].T
        qT[3, :] = 1.0
        Fc = np.zeros((CPAD, CIN), np.float32)
        Fc[:CSH] = conv_point_features[c * CSH:(c + 1) * CSH]
        r_rows, rT, F = frames[f]
        in_maps.append(dict(
            qT=qT, q_rows=q_rows, rT=rT, r_rows=r_rows, F=F, Fc=Fc, ident=ident))
    return in_maps


def _prep_weights(inp):
    g = lambda k: np.asarray(inp[k], np.float32)
    cls_s = g("cls_bn_g") / np.sqrt(g("cls_bn_v") + np.float32(EPS_BN_CLS))
    cls_b = g("cls_bn_b") - g("cls_bn_m") * cls_s
    al_s = g("al_bn_g") / np.sqrt(g("al_bn_v") + np.float32(EPS_BN_ALIGN))
    al_b = (g("al_b") - g("al_bn_m")) * al_s + g("al_bn_b")
    out_s = g("out_bn_g") / np.sqrt(g("out_bn_v") + np.float32(EPS_BN_CLS))
    out_b = g("out_bn_b") - g("out_bn_m") * out_s
    wv = np.zeros((64, 8), np.float32)
    wv[:, 0] = cls_s
    wv[:, 1] = cls_b
    wv[:NCLS, 2] = g("cls_b2")
    wv[:, 3] = al_s
    wv[:, 4] = al_b
    wv[:, 5] = out_s
    wv[:, 6] = out_b
    wv[:NCLS, 7] = g("out_b2")
    return dict(
        cls_w1=g("cls_w1"), cls_w2=g("cls_w2"), al_w=g("al_w"),
        out_w1=g("out_w1"), out_w2=g("out_w2"), wv=wv)


def kernel(conv_point_features, conv_point_coords, points,
           cls_w1, cls_bn_g, cls_bn_b, cls_bn_m, cls_bn_v, cls_w2, cls_b2,
           al_w, al_b, al_bn_g, al_bn_b, al_bn_m, al_bn_v,
           out_w1, out_bn_g, out_bn_b, out_bn_m, out_bn_v, out_w2, out_b2,
           batch_size, _trace=False):
    from concourse.bass_utils import run_bass_kernel_spmd

    inp = dict(
        cls_w1=cls_w1, cls_bn_g=cls_bn_g, cls_bn_b=cls_bn_b, cls_bn_m=cls_bn_m,
        cls_bn_v=cls_bn_v, cls_w2=cls_w2, cls_b2=cls_b2, al_w=al_w, al_b=al_b,
        al_bn_g=al_bn_g, al_bn_b=al_bn_b, al_bn_m=al_bn_m, al_bn_v=al_bn_v,
        out_w1=out_w1, out_bn_g=out_bn_g, out_bn_b=out_bn_b, out_bn_m=out_bn_m,
        out_bn_v=out_bn_v, out_w2=out_w2, out_b2=out_b2)
    wmap = _prep_weights(inp)
    in_maps = _prep_in_maps(
        np.asarray(conv_point_features), np.asarray(conv_point_coords),
        np.asarray(points))
    for m in in_maps:
        m.update(wmap)

    nc = _get_nc()
    res = run_bass_kernel_spmd(nc, in_maps, list(range(NCORES)), trace=_trace)

    conv = np.concatenate(
        [res.results[c]["conv_logits"][:CSH] for c in range(NCORES)], 0)
    outl = np.concatenate(
        [res.results[c]["out_logits"][:QSH] for c in range(NCORES)], 0)
    if _trace:
        return (conv, outl), res
    return conv, outl
